# revision 10
# baseline (speedup 1.0000x reference)
"""DbrxAttention (B=1, S=2048, D=6144, 48 q heads / 8 kv heads, rope, causal)
on 8 Trainium2 NeuronCores.

Sharding: tensor-parallel across heads. Core c owns q heads [6c, 6c+6) and kv
head c. Wqkv output dim and Wout input dim are sharded; a ReduceScatter after
out_proj sums the partial outputs, and the host concatenates the row-shards.

v2 restructure (from trace analysis of the v1 kernel):
- The v1 kernel left the PE idle 4-19us at a time inside the softmax windows
  (rope-chain latency at stage-2 entry, then per-head softmax chains); every
  gap >3.4us re-throttles the PE clock to 1.2GHz (HAM), so matmuls averaged
  289ns instead of 216ns.
- Stage 1 is now TWO passes of 4 PSUM banks each: pass A computes {k, v, q0,
  q1}, pass B {q2..q5}. Rope for k/q0/q1 and the v transpose run on
  vector/DMA *during* pass B's matmuls, so scores start immediately after
  pass B with zero rope-latency stall.
- ALL psum tiles (stage-1 accumulators, score tiles, PV, out-proj) come from
  one 8-buffer ring of [128,512] f32 tiles = the 8 physical banks. Ring
  order = emission order, so WAR dependencies pipeline 8 deep with no
  pool-scope barriers between stages.
- hidden activations are per-chunk resident in SBUF (4 quarter-tiles,
  5-deep ring = one-quarter cross-chunk prefetch), required for the two-pass
  contraction re-read. Weights stream per pass from pass-major restaged DRAM
  in 512KB blocks (4 kt per DMA trigger); out-proj weights stream per chunk
  in 12 x 768KB blocks; outT stores are batched 4 dm-tiles per trigger.
  This cuts the ~650ns-per-trigger GpSimd issue cost ~4x.
- The ReduceScatter for chunk qc-1 is emitted after chunk qc's last probsT
  transpose: its sync-queue completion-wait then sits where the next sync
  consumer (chunk qc+1's v transpose) is ~85us away, and the RS itself
  overlaps stage 3(qc) + pass A(qc+1).

Numerics identical to v1: f16 matmul operands, fp32 PSUM, fp32 softmax with
constant shift exp(s-12), fp32 normalize before the f16 cast, f16 rope.
"""

import numpy as np

N_CORES = 8
S = 2048
D = 6144
HD = 128
NQH = 6                 # q heads per core
P = 128
NKT = S // P            # 16 key tiles
NQC = 4                 # q chunks
QCW = S // NQC          # 512
DT = D // P             # 48 d-model tiles
SCALE = HD ** -0.5
CAP = 12.0              # softmax constant shift
CLIP = 8.0

_cached_nc = None


def _build_nc():
    import concourse.mybir as mybir
    import concourse.tile as tile
    from concourse import bacc

    f16, f32 = mybir.dt.float16, mybir.dt.float32
    add_op = mybir.AluOpType.add
    mult_op = mybir.AluOpType.mult
    min_op = mybir.AluOpType.min
    max_op = mybir.AluOpType.max
    X = mybir.AxisListType.X
    Exp = mybir.ActivationFunctionType.Exp

    nc = bacc.Bacc("TRN2", target_bir_lowering=False, debug=False,
                   num_devices=N_CORES)

    # hiddenQ[qc, b, p, j, c] = hiddenT[128*(12b+j)+p, 512qc+c]
    hiddenQ = nc.dram_tensor("hiddenQ", [NQC, 4, P, 12, QCW], f16,
                             kind="ExternalInput").ap()
    # wqkvA/B[blk, p, j, c]: pass-major weight blocks, 4 kt per block,
    # c = 4 m-slices of 128 (A: k,v,q0,q1 / B: q2..q5)
    wqkvA = nc.dram_tensor("wqkvA", [12, P, 4, 512], f16,
                           kind="ExternalInput").ap()
    wqkvB = nc.dram_tensor("wqkvB", [12, P, 4, 512], f16,
                           kind="ExternalInput").ap()
    # woG[g, p, h, c] = woutT[128h+p, 512g+c]
    woG = nc.dram_tensor("woG", [12, P, NQH, 512], f16,
                         kind="ExternalInput").ap()
    ccq = nc.dram_tensor("ccq", [P, S], f16, kind="ExternalInput").ap()
    ssq = nc.dram_tensor("ssq", [P, S], f16, kind="ExternalInput").ap()
    cck = nc.dram_tensor("cck", [P, S], f16, kind="ExternalInput").ap()
    ssk = nc.dram_tensor("ssk", [P, S], f16, kind="ExternalInput").ap()
    ident = nc.dram_tensor("ident", [P, P], f16, kind="ExternalInput").ap()
    maskd = nc.dram_tensor("maskd", [P, P], f16, kind="ExternalInput").ap()
    outs = [nc.dram_tensor(f"out{g}", [D, QCW], f16,
                           kind="ExternalOutput").ap() for g in range(NQC)]

    with tile.TileContext(nc) as tc:
        with (
            tc.tile_pool(name="const", bufs=1) as const,
            tc.tile_pool(name="kv", bufs=1) as kvp,
            tc.tile_pool(name="stream", bufs=1) as stream,
            tc.tile_pool(name="work", bufs=1) as work,
            tc.tile_pool(name="stats", bufs=1) as stats,
            tc.tile_pool(name="ps", bufs=1, space="PSUM") as psp,
        ):
            ident_sb = const.tile([P, P], f16, tag="ident")
            nc.sync.dma_start(ident_sb[:], ident[:])
            maskd_sb = const.tile([P, P], f16, tag="maskd")
            nc.sync.dma_start(maskd_sb[:], maskd[:])
            negcap = const.tile([P, 1], f32, tag="negcap")
            nc.vector.memset(negcap[:], -CAP)
            # pre-warm the exp table so the 2.7us ACT_TABLE_LOAD is off the
            # chunk-0 softmax critical path
            warm = stats.tile([P, 1], f32, tag="warm", name="warm")
            nc.scalar.activation(warm[:], negcap[:], Exp, scale=1.0)

            k_sb = kvp.tile([P, S], f16, tag="k_sb")
            v_sb = kvp.tile([P, NKT, P], f16, tag="v_sb")

            def ring(nm):
                return psp.tile([P, QCW], f32, tag="pb", bufs=8, name=nm)

            for qc in range(NQC):
                cs = slice(QCW * qc, QCW * (qc + 1))

                # ---- chunk-resident hidden quarters + rope tables
                def hq_load(b):
                    hq = stream.tile([P, 12, QCW], f16, tag="hq", bufs=4,
                                     name=f"hq{b}")
                    nc.gpsimd.dma_start(hq[:], hiddenQ[qc, b])
                    return hq

                hqs = [hq_load(0)]

                def hrhs(kt):
                    return hqs[kt // 12][:, kt % 12, :]

                q_h = [work.tile([P, QCW], f16, tag=f"qh{m}", bufs=2,
                                 name=f"q{m}") for m in range(NQH)]

                # ---- rope helpers (clip runs ahead of its mult chain)
                def emit_clip_v(ps_v):
                    vT = work.tile([P, QCW], f16, tag="vT", bufs=1,
                                   name="vT")
                    nc.vector.tensor_scalar(
                        vT[:], ps_v[:], CLIP, -CLIP, min_op, max_op)
                    nc.sync.dma_start_transpose(
                        v_sb[:, 4 * qc:4 * (qc + 1), :], vT[:])

                def emit_clip(ps_m):
                    a_t = work.tile([P, QCW], f16, tag="ropeA", bufs=3,
                                    name="a_t")
                    nc.vector.tensor_scalar(
                        a_t[:], ps_m[:], CLIP, -CLIP, min_op, max_op)
                    b_t = work.tile([P, QCW], f16, tag="ropeB", bufs=3,
                                    name="b_t")
                    nc.gpsimd.dma_start(b_t[0:64, :], a_t[64:128, :])
                    nc.gpsimd.dma_start(b_t[64:128, :], a_t[0:64, :])
                    return (a_t, b_t)

                def emit_chain(m, ab):
                    # m == -1 means the k head
                    a_t, b_t = ab
                    cc_t = tabs["cck"] if m < 0 else tabs["ccq"]
                    ss_t = tabs["ssk"] if m < 0 else tabs["ssq"]
                    e_t = work.tile([P, QCW], f16, tag="ropeE", bufs=2,
                                    name="e_t")
                    nc.vector.tensor_tensor(e_t[:], a_t[:], cc_t[:], mult_op)
                    nc.vector.tensor_tensor(b_t[:], b_t[:], ss_t[:], mult_op)
                    dst = k_sb[:, cs] if m < 0 else q_h[m][:]
                    nc.vector.tensor_tensor(dst, e_t[:], b_t[:], add_op)

                # ---- stage 1 pass A: {k, v, q0, q1}
                psA = [ring(f"A{i}") for i in range(4)]
                tabs = {}
                for blk in range(12):
                    wa_t = stream.tile([P, 4, 512], f16, tag="wa", bufs=2,
                                       name="wa_t")
                    nc.gpsimd.dma_start(wa_t[:], wqkvA[blk])
                    if blk in (1, 4, 7):
                        hqs.append(hq_load(len(hqs)))
                    elif blk == 8:
                        for nm, tsrc in (("ccq", ccq), ("ssq", ssq),
                                         ("cck", cck), ("ssk", ssk)):
                            t = stream.tile([P, QCW], f16, tag=nm, bufs=1,
                                            name=nm)
                            nc.gpsimd.dma_start(t[:], tsrc[:, cs])
                            tabs[nm] = t
                    for j in range(4):
                        kt = 4 * blk + j
                        for mi in range(4):
                            nc.tensor.matmul(
                                psA[mi][:], wa_t[:, j, P * mi:P * (mi + 1)],
                                hrhs(kt),
                                start=(kt == 0), stop=(kt == DT - 1))
                # pass-B weight triggers go on the gpsimd queue BEFORE the
                # rope work so the first blocks prefetch during pass A;
                # later triggers self-throttle on the wb ring.
                wb_ts = []
                for blk in range(6):
                    wb_t = stream.tile([P, 4, 512], f16, tag="wb", bufs=4,
                                       name="wb_t")
                    nc.gpsimd.dma_start(wb_t[:], wqkvB[blk])
                    wb_ts.append(wb_t)
                # drain A: clips first, then chains (k, q0, q1); v transpose
                ab_k = emit_clip(psA[0])
                emit_clip_v(psA[1])
                ab_q0 = emit_clip(psA[2])
                ab_q1 = emit_clip(psA[3])
                emit_chain(-1, ab_k)
                emit_chain(0, ab_q0)
                emit_chain(1, ab_q1)
                for blk in range(6, 12):
                    wb_t = stream.tile([P, 4, 512], f16, tag="wb", bufs=4,
                                       name="wb_t")
                    nc.gpsimd.dma_start(wb_t[:], wqkvB[blk])
                    wb_ts.append(wb_t)

                # ---- stage 1 pass B: {q2..q5}  (rope A runs under this)
                psB = [ring(f"B{i}") for i in range(4)]
                for blk in range(12):
                    wb_t = wb_ts[blk]
                    for j in range(4):
                        kt = 4 * blk + j
                        for mi in range(4):
                            nc.tensor.matmul(
                                psB[mi][:], wb_t[:, j, P * mi:P * (mi + 1)],
                                hrhs(kt),
                                start=(kt == 0), stop=(kt == DT - 1))
                abs_b = [emit_clip(psB[i]) for i in range(4)]
                for i in range(4):
                    emit_chain(2 + i, abs_b[i])

                # ---- stage 2: scores + softmax, PV lagging one head so the
                # PE never waits more than one softmax tail (and the probsT
                # bufs=2 ring stays acyclic across engines)
                njt = 4 * (qc + 1)
                attnT = work.tile([P, NQH, QCW], f16, tag="attnT", bufs=1,
                                  name="attnT")

                def emit_pv(h, probsT):
                    ps_pv = ring("pv")
                    for j in range(njt):
                        nc.tensor.matmul(
                            ps_pv[:], v_sb[:, j, :], probsT[:, j, :],
                            start=(j == 0), stop=(j == njt - 1))
                    nc.vector.tensor_copy(attnT[:, h, :], ps_pv[:])

                probsTs = {}
                for h in range(NQH):
                    probsT = work.tile([P, NKT, QCW], f16, tag="probsT",
                                       bufs=3, name="probsT")
                    probsTs[h] = probsT
                    for jl in range(1, 4):
                        nc.vector.memset(
                            probsT[:, 4 * qc + jl, :P * jl], 0.0)
                    for il in range(4):
                        i = 4 * qc + il
                        L = P * (i + 1)
                        nkc = (L + 511) // 512
                        s_all = stats.tile([P, 4], f32, tag="s_all",
                                           bufs=3, name="s_all")
                        probs16 = work.tile([P, S], f16, tag="probs16",
                                            bufs=4, name="probs16")
                        pscs = []
                        for kc in range(nkc):
                            n = min(512, L - 512 * kc)
                            last = kc == nkc - 1
                            psc = ring("psc")
                            nc.tensor.matmul(
                                psc[:, :n],
                                q_h[h][:, P * il:P * (il + 1)],
                                k_sb[:, 512 * kc:512 * kc + n],
                                start=True, stop=not last)
                            if last:
                                nc.tensor.matmul(
                                    psc[:, n - P:n], ident_sb[:],
                                    maskd_sb[:], start=False, stop=True)
                            pscs.append((psc, n, kc))
                        p32s = []
                        for psc, n, kc in pscs:
                            p32 = work.tile([P, 512], f32, tag="p32",
                                            bufs=4, name="p32")
                            nc.scalar.activation(
                                p32[:, :n],
                                psc[:, :n], Exp, bias=negcap[:],
                                scale=1.0,
                                accum_out=s_all[:, kc:kc + 1])
                            p32s.append((p32, n, kc))
                        ssum = stats.tile([P, 1], f32, tag="ssum",
                                          bufs=3, name="ssum")
                        nc.vector.reduce_sum(ssum[:], s_all[:, :nkc],
                                             axis=X)
                        rcp = stats.tile([P, 1], f32, tag="rcp",
                                         bufs=3, name="rcp")
                        nc.vector.reciprocal(rcp[:], ssum[:])
                        for p32, n, kc in p32s:
                            nc.vector.tensor_scalar_mul(
                                probs16[:, 512 * kc:512 * kc + n],
                                p32[:, :n], rcp[:])
                        teng = nc.sync if il % 2 == 0 else nc.scalar
                        teng.dma_start_transpose(
                            probsT[:, :i + 1, P * il:P * (il + 1)],
                            probs16[:, :L])
                    if h >= 2:
                        emit_pv(h - 2, probsTs[h - 2])
                emit_pv(NQH - 2, probsTs[NQH - 2])
                emit_pv(NQH - 1, probsTs[NQH - 1])

                # ---- stage 3: out-proj, batched stores direct to output
                outT_qc = outs[qc]
                for g in range(12):
                    wo_t = stream.tile([P, NQH, 512], f16, tag="wo", bufs=2,
                                       name="wo_t")
                    nc.gpsimd.dma_start(wo_t[:], woG[g])
                    ot = work.tile([P, 4, QCW], f16, tag="ot", bufs=2,
                                   name="ot")
                    for i in range(4):
                        pso = ring("pso")
                        for h6 in range(NQH):
                            nc.tensor.matmul(
                                pso[:],
                                wo_t[:, h6, P * i:P * (i + 1)],
                                attnT[:, h6, :],
                                start=(h6 == 0), stop=(h6 == NQH - 1))
                        nc.scalar.copy(ot[:, i, :], pso[:])
                    nc.gpsimd.dma_start(
                        outT_qc[512 * g:512 * (g + 1), :].rearrange(
                            "(i p) c -> p i c", p=P),
                        ot[:])

    nc.compile()
    return nc


def _get_nc():
    global _cached_nc
    if _cached_nc is None:
        _cached_nc = _build_nc()
    return _cached_nc


def kernel(**inputs):
    from concourse.bass_utils import run_bass_kernel_spmd

    hs = np.asarray(inputs["hidden_states"])[0].astype(np.float32)   # [S, D]
    Wqkv = np.asarray(inputs["Wqkv"]).astype(np.float32)             # [8192, D]
    Wout = np.asarray(inputs["Wout"]).astype(np.float32)             # [D, D]
    pos = np.asarray(inputs["position_ids"])[0]

    f16 = np.float16
    hiddenT = np.ascontiguousarray(hs.T).astype(f16)                 # [D, S]
    WT = Wqkv.T.astype(f16)                                          # [D, 8192]
    WoT = Wout.T.astype(f16)                                         # [D, D]

    # hiddenQ[qc, b, p, j, c] = hiddenT[128*(12b+j)+p, 512qc+c]
    hiddenQ = np.ascontiguousarray(
        hiddenT.reshape(4, 12, P, 4, QCW).transpose(3, 0, 2, 1, 4)
    )  # [qc, b, p, j, c]

    half = HD // 2
    inv = (1.0 / (500000.0 ** (np.arange(half, dtype=np.float32) * 2.0 / HD)))
    ang = pos.astype(np.float32)[:, None] * inv[None, :].astype(np.float32)
    cos = np.cos(ang).T.astype(np.float32)                           # [64, S]
    sin = np.sin(ang).T.astype(np.float32)
    cc = np.concatenate([cos, cos], axis=0)                          # [128, S]
    ss = np.concatenate([-sin, sin], axis=0)
    ccq = np.ascontiguousarray((cc * SCALE).astype(f16))
    ssq = np.ascontiguousarray((ss * SCALE).astype(f16))
    cck = np.ascontiguousarray(cc.astype(f16))
    ssk = np.ascontiguousarray(ss.astype(f16))
    idx = np.arange(P)
    identm = np.eye(P, dtype=np.float16)
    maskdm = np.where(idx[None, :] > idx[:, None], -60000.0, 0.0).astype(np.float16)

    in_maps = []
    for c in range(N_CORES):
        # per-core wqkv columns, pass-major: A = [k, v, q0, q1], B = [q2..q5]
        kcol = WT[:, D + P * c:D + P * (c + 1)]
        vcol = WT[:, D + 1024 + P * c:D + 1024 + P * (c + 1)]
        qcols = [WT[:, 768 * c + P * m:768 * c + P * (m + 1)]
                 for m in range(6)]
        wA = np.concatenate([kcol, vcol, qcols[0], qcols[1]], axis=1)
        wB = np.concatenate(qcols[2:6], axis=1)
        # [blk, p, j, c]: row 128*(4blk+j)+p
        wqkvA = np.ascontiguousarray(
            wA.reshape(12, 4, P, 512).transpose(0, 2, 1, 3))
        wqkvB = np.ascontiguousarray(
            wB.reshape(12, 4, P, 512).transpose(0, 2, 1, 3))
        # woG[g, p, h, c] = WoT[768c + 128h + p, 512g + c]
        wo = WoT[768 * c:768 * (c + 1), :]                           # [768, D]
        woGm = np.ascontiguousarray(
            wo.reshape(NQH, P, 12, 512).transpose(2, 1, 0, 3))
        in_maps.append(dict(hiddenQ=hiddenQ, wqkvA=wqkvA, wqkvB=wqkvB,
                            woG=woGm, ccq=ccq, ssq=ssq, cck=cck, ssk=ssk,
                            ident=identm, maskd=maskdm))

    nc = _get_nc()
    res = run_bass_kernel_spmd(nc, in_maps, core_ids=list(range(N_CORES)))
    kernel._last_results = res

    # unshard: each core returns its partial out-projection (contraction over
    # its 6 heads); the full output is the sum of the 8 partials.
    outT = np.zeros((D, S), np.float32)
    for qc in range(NQC):
        acc = np.zeros((D, QCW), np.float32)
        for c in range(N_CORES):
            acc += res.results[c][f"out{qc}"].astype(np.float32)
        outT[:, QCW * qc:QCW * (qc + 1)] = acc
    return np.ascontiguousarray(outT.T)[None]


# revision 12
# speedup vs baseline: 1.0512x; 1.0512x over previous
"""DbrxAttention (B=1, S=2048, D=6144, 48 q heads / 8 kv heads, rope, causal)
on 8 Trainium2 NeuronCores.

Sharding: tensor-parallel across heads. Core c owns q heads [6c, 6c+6) and kv
head c. Wqkv output dim and Wout input dim are sharded; a ReduceScatter after
out_proj sums the partial outputs, and the host concatenates the row-shards.

v2 restructure (from trace analysis of the v1 kernel):
- The v1 kernel left the PE idle 4-19us at a time inside the softmax windows
  (rope-chain latency at stage-2 entry, then per-head softmax chains); every
  gap >3.4us re-throttles the PE clock to 1.2GHz (HAM), so matmuls averaged
  289ns instead of 216ns.
- Stage 1 is now TWO passes of 4 PSUM banks each: pass A computes {k, v, q0,
  q1}, pass B {q2..q5}. Rope for k/q0/q1 and the v transpose run on
  vector/DMA *during* pass B's matmuls, so scores start immediately after
  pass B with zero rope-latency stall.
- ALL psum tiles (stage-1 accumulators, score tiles, PV, out-proj) come from
  one 8-buffer ring of [128,512] f32 tiles = the 8 physical banks. Ring
  order = emission order, so WAR dependencies pipeline 8 deep with no
  pool-scope barriers between stages.
- hidden activations are per-chunk resident in SBUF (4 quarter-tiles,
  5-deep ring = one-quarter cross-chunk prefetch), required for the two-pass
  contraction re-read. Weights stream per pass from pass-major restaged DRAM
  in 512KB blocks (4 kt per DMA trigger); out-proj weights stream per chunk
  in 12 x 768KB blocks; outT stores are batched 4 dm-tiles per trigger.
  This cuts the ~650ns-per-trigger GpSimd issue cost ~4x.
- The ReduceScatter for chunk qc-1 is emitted after chunk qc's last probsT
  transpose: its sync-queue completion-wait then sits where the next sync
  consumer (chunk qc+1's v transpose) is ~85us away, and the RS itself
  overlaps stage 3(qc) + pass A(qc+1).

Numerics identical to v1: f16 matmul operands, fp32 PSUM, fp32 softmax with
constant shift exp(s-12), fp32 normalize before the f16 cast, f16 rope.
"""

import numpy as np

N_CORES = 8
S = 2048
D = 6144
HD = 128
NQH = 6                 # q heads per core
P = 128
NKT = S // P            # 16 key tiles
NQC = 4                 # q chunks
QCW = S // NQC          # 512
DT = D // P             # 48 d-model tiles
SCALE = HD ** -0.5
CAP = 12.0              # softmax constant shift
CLIP = 8.0

_cached_nc = None


def _build_nc():
    import concourse.mybir as mybir
    import concourse.tile as tile
    from concourse import bacc

    f16, f32 = mybir.dt.float16, mybir.dt.float32
    add_op = mybir.AluOpType.add
    mult_op = mybir.AluOpType.mult
    min_op = mybir.AluOpType.min
    max_op = mybir.AluOpType.max
    X = mybir.AxisListType.X
    Exp = mybir.ActivationFunctionType.Exp

    nc = bacc.Bacc("TRN2", target_bir_lowering=False, debug=False,
                   num_devices=N_CORES)

    # hiddenQ[qc, b, p, j, c] = hiddenT[128*(12b+j)+p, 512qc+c]
    hiddenQ = nc.dram_tensor("hiddenQ", [NQC, 4, P, 12, QCW], f16,
                             kind="ExternalInput").ap()
    # wqkvA/B[blk, p, j, c]: pass-major weight blocks, 4 kt per block,
    # c = 4 m-slices of 128 (A: k,v,q0,q1 / B: q2..q5)
    wqkvA = nc.dram_tensor("wqkvA", [12, P, 4, 512], f16,
                           kind="ExternalInput").ap()
    wqkvB = nc.dram_tensor("wqkvB", [12, P, 4, 512], f16,
                           kind="ExternalInput").ap()
    # woG[g, p, h, c] = woutT[128h+p, 512g+c]
    woG = nc.dram_tensor("woG", [12, P, NQH, 512], f16,
                         kind="ExternalInput").ap()
    ccq = nc.dram_tensor("ccq", [P, S], f16, kind="ExternalInput").ap()
    ssq = nc.dram_tensor("ssq", [P, S], f16, kind="ExternalInput").ap()
    cck = nc.dram_tensor("cck", [P, S], f16, kind="ExternalInput").ap()
    ssk = nc.dram_tensor("ssk", [P, S], f16, kind="ExternalInput").ap()
    ident = nc.dram_tensor("ident", [P, P], f16, kind="ExternalInput").ap()
    maskd = nc.dram_tensor("maskd", [P, P], f16, kind="ExternalInput").ap()
    outs = [nc.dram_tensor(f"out{g}", [D, QCW], f16,
                           kind="ExternalOutput").ap() for g in range(NQC)]

    with tile.TileContext(nc) as tc:
        with (
            tc.tile_pool(name="const", bufs=1) as const,
            tc.tile_pool(name="kv", bufs=1) as kvp,
            tc.tile_pool(name="stream", bufs=1) as stream,
            tc.tile_pool(name="work", bufs=1) as work,
            tc.tile_pool(name="stats", bufs=1) as stats,
            tc.tile_pool(name="ps", bufs=1, space="PSUM") as psp,
        ):
            ident_sb = const.tile([P, P], f16, tag="ident")
            nc.sync.dma_start(ident_sb[:], ident[:])
            maskd_sb = const.tile([P, P], f16, tag="maskd")
            nc.sync.dma_start(maskd_sb[:], maskd[:])
            negcap = const.tile([P, 1], f32, tag="negcap")
            nc.vector.memset(negcap[:], -CAP)
            # pre-warm the exp table so the 2.7us ACT_TABLE_LOAD is off the
            # chunk-0 softmax critical path
            warm = stats.tile([P, 1], f32, tag="warm", name="warm")
            nc.scalar.activation(warm[:], negcap[:], Exp, scale=1.0)

            k_sb = kvp.tile([P, S], f16, tag="k_sb")
            v_sb = kvp.tile([P, NKT, P], f16, tag="v_sb")

            def ring(nm):
                return psp.tile([P, QCW], f32, tag="pb", bufs=8, name=nm)

            for qc in range(NQC):
                cs = slice(QCW * qc, QCW * (qc + 1))

                # ---- chunk-resident hidden quarters + rope tables
                def hq_load(b):
                    hq = stream.tile([P, 12, QCW], f16, tag="hq", bufs=4,
                                     name=f"hq{b}")
                    nc.gpsimd.dma_start(hq[:], hiddenQ[qc, b])
                    return hq

                hqs = [hq_load(0)]

                def hrhs(kt):
                    return hqs[kt // 12][:, kt % 12, :]

                q_h = [work.tile([P, QCW], f16, tag=f"qh{m}", bufs=2,
                                 name=f"q{m}") for m in range(NQH)]

                # ---- rope helpers (clip runs ahead of its mult chain)
                def emit_clip_v(ps_v):
                    vT = work.tile([P, QCW], f16, tag="vT", bufs=1,
                                   name="vT")
                    nc.vector.tensor_scalar(
                        vT[:], ps_v[:], CLIP, -CLIP, min_op, max_op)
                    nc.sync.dma_start_transpose(
                        v_sb[:, 4 * qc:4 * (qc + 1), :], vT[:])

                def emit_clip(ps_m):
                    a_t = work.tile([P, QCW], f16, tag="ropeA", bufs=3,
                                    name="a_t")
                    nc.vector.tensor_scalar(
                        a_t[:], ps_m[:], CLIP, -CLIP, min_op, max_op)
                    b_t = work.tile([P, QCW], f16, tag="ropeB", bufs=3,
                                    name="b_t")
                    nc.gpsimd.dma_start(b_t[0:64, :], a_t[64:128, :])
                    nc.gpsimd.dma_start(b_t[64:128, :], a_t[0:64, :])
                    return (a_t, b_t)

                def emit_chain(m, ab):
                    # m == -1 means the k head
                    a_t, b_t = ab
                    cc_t = tabs["cck"] if m < 0 else tabs["ccq"]
                    ss_t = tabs["ssk"] if m < 0 else tabs["ssq"]
                    e_t = work.tile([P, QCW], f16, tag="ropeE", bufs=2,
                                    name="e_t")
                    nc.vector.tensor_tensor(e_t[:], a_t[:], cc_t[:], mult_op)
                    nc.vector.tensor_tensor(b_t[:], b_t[:], ss_t[:], mult_op)
                    dst = k_sb[:, cs] if m < 0 else q_h[m][:]
                    nc.vector.tensor_tensor(dst, e_t[:], b_t[:], add_op)

                # ---- stage 1 pass A: {k, v, q0, q1}
                psA = [ring(f"A{i}") for i in range(4)]
                tabs = {}
                for blk in range(12):
                    wa_t = stream.tile([P, 4, 512], f16, tag="wa", bufs=2,
                                       name="wa_t")
                    nc.gpsimd.dma_start(wa_t[:], wqkvA[blk])
                    if blk in (1, 4, 7):
                        hqs.append(hq_load(len(hqs)))
                    elif blk == 8:
                        for nm, tsrc in (("ccq", ccq), ("ssq", ssq),
                                         ("cck", cck), ("ssk", ssk)):
                            t = stream.tile([P, QCW], f16, tag=nm, bufs=2,
                                            name=nm)
                            nc.gpsimd.dma_start(t[:], tsrc[:, cs])
                            tabs[nm] = t
                    for j in range(4):
                        kt = 4 * blk + j
                        for mi in range(4):
                            nc.tensor.matmul(
                                psA[mi][:], wa_t[:, j, P * mi:P * (mi + 1)],
                                hrhs(kt),
                                start=(kt == 0), stop=(kt == DT - 1))
                # pass-B weight triggers go on the gpsimd queue BEFORE the
                # rope work so the first blocks prefetch during pass A;
                # later triggers self-throttle on the wb ring.
                wb_ts = []
                for blk in range(6):
                    wb_t = stream.tile([P, 4, 512], f16, tag="wb", bufs=4,
                                       name="wb_t")
                    nc.gpsimd.dma_start(wb_t[:], wqkvB[blk])
                    wb_ts.append(wb_t)
                # drain A: clips first, then chains (k, q0, q1); v transpose
                ab_k = emit_clip(psA[0])
                emit_clip_v(psA[1])
                ab_q0 = emit_clip(psA[2])
                ab_q1 = emit_clip(psA[3])
                emit_chain(-1, ab_k)
                emit_chain(0, ab_q0)
                emit_chain(1, ab_q1)
                for blk in range(6, 12):
                    wb_t = stream.tile([P, 4, 512], f16, tag="wb", bufs=4,
                                       name="wb_t")
                    nc.gpsimd.dma_start(wb_t[:], wqkvB[blk])
                    wb_ts.append(wb_t)

                # ---- stage 1 pass B: {q2..q5}  (rope A runs under this)
                psB = [ring(f"B{i}") for i in range(4)]
                for blk in range(12):
                    wb_t = wb_ts[blk]
                    for j in range(4):
                        kt = 4 * blk + j
                        for mi in range(4):
                            nc.tensor.matmul(
                                psB[mi][:], wb_t[:, j, P * mi:P * (mi + 1)],
                                hrhs(kt),
                                start=(kt == 0), stop=(kt == DT - 1))
                abs_b = [emit_clip(psB[i]) for i in range(4)]
                for i in range(4):
                    emit_chain(2 + i, abs_b[i])

                # ---- stage 2: scores + softmax, PV lagging one head so the
                # PE never waits more than one softmax tail (and the probsT
                # bufs=2 ring stays acyclic across engines)
                njt = 4 * (qc + 1)
                attnT = work.tile([P, NQH, QCW], f16, tag="attnT", bufs=1,
                                  name="attnT")

                def emit_pv(h, probsT):
                    ps_pv = ring("pv")
                    for j in range(njt):
                        nc.tensor.matmul(
                            ps_pv[:], v_sb[:, j, :], probsT[:, j, :],
                            start=(j == 0), stop=(j == njt - 1))
                    nc.vector.tensor_copy(attnT[:, h, :], ps_pv[:])

                probsTs = {}
                for h in range(NQH):
                    probsT = work.tile([P, NKT, QCW], f16, tag="probsT",
                                       bufs=2, name="probsT")
                    probsTs[h] = probsT
                    for jl in range(1, 4):
                        nc.vector.memset(
                            probsT[:, 4 * qc + jl, :P * jl], 0.0)
                    for il in range(4):
                        i = 4 * qc + il
                        L = P * (i + 1)
                        nkc = (L + 511) // 512
                        s_all = stats.tile([P, 4], f32, tag="s_all",
                                           bufs=3, name="s_all")
                        probs16 = work.tile([P, S], f16, tag="probs16",
                                            bufs=5, name="probs16")
                        pscs = []
                        for kc in range(nkc):
                            n = min(512, L - 512 * kc)
                            last = kc == nkc - 1
                            psc = ring("psc")
                            nc.tensor.matmul(
                                psc[:, :n],
                                q_h[h][:, P * il:P * (il + 1)],
                                k_sb[:, 512 * kc:512 * kc + n],
                                start=True, stop=not last)
                            if last:
                                nc.tensor.matmul(
                                    psc[:, n - P:n], ident_sb[:],
                                    maskd_sb[:], start=False, stop=True)
                            pscs.append((psc, n, kc))
                        p32s = []
                        for psc, n, kc in pscs:
                            p32 = work.tile([P, 512], f32, tag="p32",
                                            bufs=6, name="p32")
                            nc.scalar.activation(
                                p32[:, :n],
                                psc[:, :n], Exp, bias=negcap[:],
                                scale=1.0,
                                accum_out=s_all[:, kc:kc + 1])
                            p32s.append((p32, n, kc))
                        ssum = stats.tile([P, 1], f32, tag="ssum",
                                          bufs=3, name="ssum")
                        nc.vector.reduce_sum(ssum[:], s_all[:, :nkc],
                                             axis=X)
                        rcp = stats.tile([P, 1], f32, tag="rcp",
                                         bufs=3, name="rcp")
                        nc.vector.reciprocal(rcp[:], ssum[:])
                        for p32, n, kc in p32s:
                            nc.vector.tensor_scalar_mul(
                                probs16[:, 512 * kc:512 * kc + n],
                                p32[:, :n], rcp[:])
                        teng = nc.sync if il % 2 == 0 else nc.scalar
                        teng.dma_start_transpose(
                            probsT[:, :i + 1, P * il:P * (il + 1)],
                            probs16[:, :L])
                    if h >= 1:
                        emit_pv(h - 1, probsTs[h - 1])
                emit_pv(NQH - 1, probsTs[NQH - 1])

                # ---- stage 3: out-proj, batched stores direct to output
                outT_qc = outs[qc]
                for g in range(12):
                    wo_t = stream.tile([P, NQH, 512], f16, tag="wo", bufs=3,
                                       name="wo_t")
                    nc.gpsimd.dma_start(wo_t[:], woG[g])
                    ot = work.tile([P, 4, QCW], f16, tag="ot", bufs=2,
                                   name="ot")
                    for i in range(4):
                        pso = ring("pso")
                        for h6 in range(NQH):
                            nc.tensor.matmul(
                                pso[:],
                                wo_t[:, h6, P * i:P * (i + 1)],
                                attnT[:, h6, :],
                                start=(h6 == 0), stop=(h6 == NQH - 1))
                        nc.scalar.copy(ot[:, i, :], pso[:])
                    nc.gpsimd.dma_start(
                        outT_qc[512 * g:512 * (g + 1), :].rearrange(
                            "(i p) c -> p i c", p=P),
                        ot[:])

    nc.compile()
    return nc


def _get_nc():
    global _cached_nc
    if _cached_nc is None:
        _cached_nc = _build_nc()
    return _cached_nc


def kernel(**inputs):
    from concourse.bass_utils import run_bass_kernel_spmd

    hs = np.asarray(inputs["hidden_states"])[0].astype(np.float32)   # [S, D]
    Wqkv = np.asarray(inputs["Wqkv"]).astype(np.float32)             # [8192, D]
    Wout = np.asarray(inputs["Wout"]).astype(np.float32)             # [D, D]
    pos = np.asarray(inputs["position_ids"])[0]

    f16 = np.float16
    hiddenT = np.ascontiguousarray(hs.T).astype(f16)                 # [D, S]
    WT = Wqkv.T.astype(f16)                                          # [D, 8192]
    WoT = Wout.T.astype(f16)                                         # [D, D]

    # hiddenQ[qc, b, p, j, c] = hiddenT[128*(12b+j)+p, 512qc+c]
    hiddenQ = np.ascontiguousarray(
        hiddenT.reshape(4, 12, P, 4, QCW).transpose(3, 0, 2, 1, 4)
    )  # [qc, b, p, j, c]

    half = HD // 2
    inv = (1.0 / (500000.0 ** (np.arange(half, dtype=np.float32) * 2.0 / HD)))
    ang = pos.astype(np.float32)[:, None] * inv[None, :].astype(np.float32)
    cos = np.cos(ang).T.astype(np.float32)                           # [64, S]
    sin = np.sin(ang).T.astype(np.float32)
    cc = np.concatenate([cos, cos], axis=0)                          # [128, S]
    ss = np.concatenate([-sin, sin], axis=0)
    ccq = np.ascontiguousarray((cc * SCALE).astype(f16))
    ssq = np.ascontiguousarray((ss * SCALE).astype(f16))
    cck = np.ascontiguousarray(cc.astype(f16))
    ssk = np.ascontiguousarray(ss.astype(f16))
    idx = np.arange(P)
    identm = np.eye(P, dtype=np.float16)
    maskdm = np.where(idx[None, :] > idx[:, None], -60000.0, 0.0).astype(np.float16)

    in_maps = []
    for c in range(N_CORES):
        # per-core wqkv columns, pass-major: A = [k, v, q0, q1], B = [q2..q5]
        kcol = WT[:, D + P * c:D + P * (c + 1)]
        vcol = WT[:, D + 1024 + P * c:D + 1024 + P * (c + 1)]
        qcols = [WT[:, 768 * c + P * m:768 * c + P * (m + 1)]
                 for m in range(6)]
        wA = np.concatenate([kcol, vcol, qcols[0], qcols[1]], axis=1)
        wB = np.concatenate(qcols[2:6], axis=1)
        # [blk, p, j, c]: row 128*(4blk+j)+p
        wqkvA = np.ascontiguousarray(
            wA.reshape(12, 4, P, 512).transpose(0, 2, 1, 3))
        wqkvB = np.ascontiguousarray(
            wB.reshape(12, 4, P, 512).transpose(0, 2, 1, 3))
        # woG[g, p, h, c] = WoT[768c + 128h + p, 512g + c]
        wo = WoT[768 * c:768 * (c + 1), :]                           # [768, D]
        woGm = np.ascontiguousarray(
            wo.reshape(NQH, P, 12, 512).transpose(2, 1, 0, 3))
        in_maps.append(dict(hiddenQ=hiddenQ, wqkvA=wqkvA, wqkvB=wqkvB,
                            woG=woGm, ccq=ccq, ssq=ssq, cck=cck, ssk=ssk,
                            ident=identm, maskd=maskdm))

    nc = _get_nc()
    res = run_bass_kernel_spmd(nc, in_maps, core_ids=list(range(N_CORES)))
    kernel._last_results = res

    # unshard: each core returns its partial out-projection (contraction over
    # its 6 heads); the full output is the sum of the 8 partials.
    outT = np.zeros((D, S), np.float32)
    for qc in range(NQC):
        acc = np.zeros((D, QCW), np.float32)
        for c in range(N_CORES):
            acc += res.results[c][f"out{qc}"].astype(np.float32)
        outT[:, QCW * qc:QCW * (qc + 1)] = acc
    return np.ascontiguousarray(outT.T)[None]


# revision 15
# speedup vs baseline: 1.0517x; 1.0005x over previous
"""DbrxAttention (B=1, S=2048, D=6144, 48 q heads / 8 kv heads, rope, causal)
on 8 Trainium2 NeuronCores.

Sharding: tensor-parallel across heads. Core c owns q heads [6c, 6c+6) and kv
head c. Wqkv output dim and Wout input dim are sharded; a ReduceScatter after
out_proj sums the partial outputs, and the host concatenates the row-shards.

v2 restructure (from trace analysis of the v1 kernel):
- The v1 kernel left the PE idle 4-19us at a time inside the softmax windows
  (rope-chain latency at stage-2 entry, then per-head softmax chains); every
  gap >3.4us re-throttles the PE clock to 1.2GHz (HAM), so matmuls averaged
  289ns instead of 216ns.
- Stage 1 is now TWO passes of 4 PSUM banks each: pass A computes {k, v, q0,
  q1}, pass B {q2..q5}. Rope for k/q0/q1 and the v transpose run on
  vector/DMA *during* pass B's matmuls, so scores start immediately after
  pass B with zero rope-latency stall.
- ALL psum tiles (stage-1 accumulators, score tiles, PV, out-proj) come from
  one 8-buffer ring of [128,512] f32 tiles = the 8 physical banks. Ring
  order = emission order, so WAR dependencies pipeline 8 deep with no
  pool-scope barriers between stages.
- hidden activations are per-chunk resident in SBUF (4 quarter-tiles,
  5-deep ring = one-quarter cross-chunk prefetch), required for the two-pass
  contraction re-read. Weights stream per pass from pass-major restaged DRAM
  in 512KB blocks (4 kt per DMA trigger); out-proj weights stream per chunk
  in 12 x 768KB blocks; outT stores are batched 4 dm-tiles per trigger.
  This cuts the ~650ns-per-trigger GpSimd issue cost ~4x.
- The ReduceScatter for chunk qc-1 is emitted after chunk qc's last probsT
  transpose: its sync-queue completion-wait then sits where the next sync
  consumer (chunk qc+1's v transpose) is ~85us away, and the RS itself
  overlaps stage 3(qc) + pass A(qc+1).

Numerics identical to v1: f16 matmul operands, fp32 PSUM, fp32 softmax with
constant shift exp(s-12), fp32 normalize before the f16 cast, f16 rope.
"""

import numpy as np

N_CORES = 8
S = 2048
D = 6144
HD = 128
NQH = 6                 # q heads per core
P = 128
NKT = S // P            # 16 key tiles
NQC = 4                 # q chunks
QCW = S // NQC          # 512
DT = D // P             # 48 d-model tiles
SCALE = HD ** -0.5
CAP = 12.0              # softmax constant shift
CLIP = 8.0

_cached_nc = None


def _build_nc():
    import concourse.mybir as mybir
    import concourse.tile as tile
    from concourse import bacc

    f16, f32 = mybir.dt.float16, mybir.dt.float32
    add_op = mybir.AluOpType.add
    mult_op = mybir.AluOpType.mult
    min_op = mybir.AluOpType.min
    max_op = mybir.AluOpType.max
    X = mybir.AxisListType.X
    Exp = mybir.ActivationFunctionType.Exp

    nc = bacc.Bacc("TRN2", target_bir_lowering=False, debug=False,
                   num_devices=N_CORES)

    # hiddenQ[qc, b, p, j, c] = hiddenT[128*(12b+j)+p, 512qc+c]
    hiddenQ = nc.dram_tensor("hiddenQ", [NQC, 4, P, 12, QCW], f16,
                             kind="ExternalInput").ap()
    # wqkvA/B[blk, p, j, c]: pass-major weight blocks, 4 kt per block,
    # c = 4 m-slices of 128 (A: k,v,q0,q1 / B: q2..q5)
    wqkvA = nc.dram_tensor("wqkvA", [12, P, 4, 512], f16,
                           kind="ExternalInput").ap()
    wqkvB = nc.dram_tensor("wqkvB", [12, P, 4, 512], f16,
                           kind="ExternalInput").ap()
    # woG[g, p, h, c] = woutT[128h+p, 512g+c]
    woG = nc.dram_tensor("woG", [12, P, NQH, 512], f16,
                         kind="ExternalInput").ap()
    ccq = nc.dram_tensor("ccq", [P, S], f16, kind="ExternalInput").ap()
    ssq = nc.dram_tensor("ssq", [P, S], f16, kind="ExternalInput").ap()
    cck = nc.dram_tensor("cck", [P, S], f16, kind="ExternalInput").ap()
    ssk = nc.dram_tensor("ssk", [P, S], f16, kind="ExternalInput").ap()
    ident = nc.dram_tensor("ident", [P, P], f16, kind="ExternalInput").ap()
    maskd = nc.dram_tensor("maskd", [P, P], f16, kind="ExternalInput").ap()
    outs = [nc.dram_tensor(f"out{g}", [D, QCW], f16,
                           kind="ExternalOutput").ap() for g in range(NQC)]

    with tile.TileContext(nc) as tc:
        with (
            tc.tile_pool(name="const", bufs=1) as const,
            tc.tile_pool(name="kv", bufs=1) as kvp,
            tc.tile_pool(name="stream", bufs=1) as stream,
            tc.tile_pool(name="work", bufs=1) as work,
            tc.tile_pool(name="stats", bufs=1) as stats,
            tc.tile_pool(name="ps", bufs=1, space="PSUM") as psp,
        ):
            ident_sb = const.tile([P, P], f16, tag="ident")
            nc.sync.dma_start(ident_sb[:], ident[:])
            maskd_sb = const.tile([P, P], f16, tag="maskd")
            nc.sync.dma_start(maskd_sb[:], maskd[:])
            negcap = const.tile([P, 1], f32, tag="negcap")
            nc.vector.memset(negcap[:], -CAP)
            # pre-warm the exp table so the 2.7us ACT_TABLE_LOAD is off the
            # chunk-0 softmax critical path
            warm = stats.tile([P, 1], f32, tag="warm", name="warm")
            nc.scalar.activation(warm[:], negcap[:], Exp, scale=1.0)

            k_sb = kvp.tile([P, S], f16, tag="k_sb")
            v_sb = kvp.tile([P, NKT, P], f16, tag="v_sb")

            def ring(nm):
                return psp.tile([P, QCW], f32, tag="pb", bufs=8, name=nm)

            for qc in range(NQC):
                cs = slice(QCW * qc, QCW * (qc + 1))

                # ---- chunk-resident hidden quarters + rope tables
                def hq_load(b):
                    hq = stream.tile([P, 12, QCW], f16, tag="hq", bufs=4,
                                     name=f"hq{b}")
                    nc.gpsimd.dma_start(hq[:], hiddenQ[qc, b])
                    return hq

                hqs = [hq_load(0)]

                def hrhs(kt):
                    return hqs[kt // 12][:, kt % 12, :]

                q_h = [work.tile([P, QCW], f16, tag=f"qh{m}", bufs=2,
                                 name=f"q{m}") for m in range(NQH)]

                # ---- rope helpers (clip runs ahead of its mult chain)
                def emit_clip_v(ps_v):
                    vT = work.tile([P, QCW], f16, tag="vT", bufs=1,
                                   name="vT")
                    nc.vector.tensor_scalar(
                        vT[:], ps_v[:], CLIP, -CLIP, min_op, max_op)
                    nc.sync.dma_start_transpose(
                        v_sb[:, 4 * qc:4 * (qc + 1), :], vT[:])

                def emit_clip(ps_m):
                    a_t = work.tile([P, QCW], f16, tag="ropeA", bufs=3,
                                    name="a_t")
                    nc.vector.tensor_scalar(
                        a_t[:], ps_m[:], CLIP, -CLIP, min_op, max_op)
                    b_t = work.tile([P, QCW], f16, tag="ropeB", bufs=3,
                                    name="b_t")
                    nc.gpsimd.dma_start(b_t[0:64, :], a_t[64:128, :])
                    nc.gpsimd.dma_start(b_t[64:128, :], a_t[0:64, :])
                    return (a_t, b_t)

                def emit_chain(m, ab):
                    # m == -1 means the k head
                    a_t, b_t = ab
                    cc_t = tabs["cck"] if m < 0 else tabs["ccq"]
                    ss_t = tabs["ssk"] if m < 0 else tabs["ssq"]
                    e_t = work.tile([P, QCW], f16, tag="ropeE", bufs=2,
                                    name="e_t")
                    nc.vector.tensor_tensor(e_t[:], a_t[:], cc_t[:], mult_op)
                    nc.vector.tensor_tensor(b_t[:], b_t[:], ss_t[:], mult_op)
                    dst = k_sb[:, cs] if m < 0 else q_h[m][:]
                    nc.vector.tensor_tensor(dst, e_t[:], b_t[:], add_op)

                # ---- stage 1 pass A: {k, v, q0, q1}
                psA = [ring(f"A{i}") for i in range(4)]
                tabs = {}
                for blk in range(12):
                    wa_t = stream.tile([P, 4, 512], f16, tag="wa", bufs=2,
                                       name="wa_t")
                    nc.gpsimd.dma_start(wa_t[:], wqkvA[blk])
                    if blk in (1, 4, 7):
                        hqs.append(hq_load(len(hqs)))
                    elif blk == 8:
                        for nm, tsrc in (("ccq", ccq), ("ssq", ssq),
                                         ("cck", cck), ("ssk", ssk)):
                            t = stream.tile([P, QCW], f16, tag=nm, bufs=2,
                                            name=nm)
                            nc.gpsimd.dma_start(t[:], tsrc[:, cs])
                            tabs[nm] = t
                    for j in range(4):
                        kt = 4 * blk + j
                        for mi in range(4):
                            nc.tensor.matmul(
                                psA[mi][:], wa_t[:, j, P * mi:P * (mi + 1)],
                                hrhs(kt),
                                start=(kt == 0), stop=(kt == DT - 1))
                # pass-B weight triggers go on the gpsimd queue BEFORE the
                # rope work so the first blocks prefetch during pass A;
                # later triggers self-throttle on the wb ring.
                wb_ts = []
                for blk in range(6):
                    wb_t = stream.tile([P, 4, 512], f16, tag="wb", bufs=4,
                                       name="wb_t")
                    nc.gpsimd.dma_start(wb_t[:], wqkvB[blk])
                    wb_ts.append(wb_t)
                # drain A: clips first, then chains (k, q0, q1); v transpose
                ab_k = emit_clip(psA[0])
                emit_clip_v(psA[1])
                ab_q0 = emit_clip(psA[2])
                ab_q1 = emit_clip(psA[3])
                emit_chain(-1, ab_k)
                emit_chain(0, ab_q0)
                emit_chain(1, ab_q1)
                for blk in range(6, 12):
                    wb_t = stream.tile([P, 4, 512], f16, tag="wb", bufs=4,
                                       name="wb_t")
                    nc.gpsimd.dma_start(wb_t[:], wqkvB[blk])
                    wb_ts.append(wb_t)

                # ---- stage 1 pass B: {q2..q5}  (rope A runs under this)
                psB = [ring(f"B{i}") for i in range(4)]
                for blk in range(12):
                    wb_t = wb_ts[blk]
                    for j in range(4):
                        kt = 4 * blk + j
                        for mi in range(4):
                            nc.tensor.matmul(
                                psB[mi][:], wb_t[:, j, P * mi:P * (mi + 1)],
                                hrhs(kt),
                                start=(kt == 0), stop=(kt == DT - 1))
                abs_b = [emit_clip(psB[i]) for i in range(4)]
                for i in range(4):
                    emit_chain(2 + i, abs_b[i])

                # ---- stage 2: scores + softmax, PV lagging one head so the
                # PE never waits more than one softmax tail (and the probsT
                # bufs=2 ring stays acyclic across engines)
                njt = 4 * (qc + 1)
                attnT = work.tile([P, NQH, QCW], f16, tag="attnT", bufs=1,
                                  name="attnT")

                def emit_pv(h, probsT):
                    ps_pv = ring("pv")
                    for j in range(njt):
                        nc.tensor.matmul(
                            ps_pv[:], v_sb[:, j, :], probsT[:, j, :],
                            start=(j == 0), stop=(j == njt - 1))
                    nc.vector.tensor_copy(attnT[:, h, :], ps_pv[:])

                probsTs = {}
                for h in range(NQH):
                    probsT = work.tile([P, NKT, QCW], f16, tag="probsT",
                                       bufs=2, name="probsT")
                    probsTs[h] = probsT
                    for jl in range(1, 4):
                        nc.vector.memset(
                            probsT[:, 4 * qc + jl, :P * jl], 0.0)
                    for il in range(4):
                        i = 4 * qc + il
                        L = P * (i + 1)
                        nkc = (L + 511) // 512
                        s_all = stats.tile([P, 4], f32, tag="s_all",
                                           bufs=3, name="s_all")
                        probs16 = work.tile([P, S], f16, tag="probs16",
                                            bufs=5, name="probs16")
                        pscs = []
                        for kc in range(nkc):
                            n = min(512, L - 512 * kc)
                            last = kc == nkc - 1
                            psc = ring("psc")
                            nc.tensor.matmul(
                                psc[:, :n],
                                q_h[h][:, P * il:P * (il + 1)],
                                k_sb[:, 512 * kc:512 * kc + n],
                                start=True, stop=not last)
                            if last:
                                nc.tensor.matmul(
                                    psc[:, n - P:n], ident_sb[:],
                                    maskd_sb[:], start=False, stop=True)
                            pscs.append((psc, n, kc))
                        p32s = []
                        for psc, n, kc in pscs:
                            p32 = work.tile([P, 512], f32, tag="p32",
                                            bufs=6, name="p32")
                            nc.scalar.activation(
                                p32[:, :n],
                                psc[:, :n], Exp, bias=negcap[:],
                                scale=1.0,
                                accum_out=s_all[:, kc:kc + 1])
                            p32s.append((p32, n, kc))
                        ssum = stats.tile([P, 1], f32, tag="ssum",
                                          bufs=3, name="ssum")
                        nc.vector.reduce_sum(ssum[:], s_all[:, :nkc],
                                             axis=X)
                        rcp = stats.tile([P, 1], f32, tag="rcp",
                                         bufs=3, name="rcp")
                        nc.vector.reciprocal(rcp[:], ssum[:])
                        for p32, n, kc in p32s:
                            nc.vector.tensor_scalar_mul(
                                probs16[:, 512 * kc:512 * kc + n],
                                p32[:, :n], rcp[:])
                        teng = nc.sync if il % 2 == 0 else nc.scalar
                        teng.dma_start_transpose(
                            probsT[:, :i + 1, P * il:P * (il + 1)],
                            probs16[:, :L])
                    if h >= 1:
                        emit_pv(h - 1, probsTs[h - 1])
                emit_pv(NQH - 1, probsTs[NQH - 1])

                # ---- stage 3: out-proj, batched stores direct to output
                outT_qc = outs[qc]
                for g in range(12):
                    wo_t = stream.tile([P, NQH, 512], f16, tag="wo", bufs=3,
                                       name="wo_t")
                    nc.gpsimd.dma_start(wo_t[:], woG[g])
                    ot = work.tile([P, 4, QCW], f16, tag="ot", bufs=2,
                                   name="ot")
                    for i in range(4):
                        pso = ring("pso")
                        for h6 in range(NQH):
                            nc.tensor.matmul(
                                pso[:],
                                wo_t[:, h6, P * i:P * (i + 1)],
                                attnT[:, h6, :],
                                start=(h6 == 0), stop=(h6 == NQH - 1))
                        nc.scalar.copy(ot[:, i, :], pso[:])
                    nc.gpsimd.dma_start(
                        outT_qc[512 * g:512 * (g + 1), :].rearrange(
                            "(i p) c -> p i c", p=P),
                        ot[:])

    nc.compile()
    return nc


def _get_nc():
    global _cached_nc
    if _cached_nc is None:
        _cached_nc = _build_nc()
    return _cached_nc


def kernel(**inputs):
    from concourse.bass_utils import run_bass_kernel_spmd

    hs = np.asarray(inputs["hidden_states"])[0].astype(np.float32)   # [S, D]
    Wqkv = np.asarray(inputs["Wqkv"]).astype(np.float32)             # [8192, D]
    Wout = np.asarray(inputs["Wout"]).astype(np.float32)             # [D, D]
    pos = np.asarray(inputs["position_ids"])[0]

    f16 = np.float16
    hiddenT = np.ascontiguousarray(hs.T).astype(f16)                 # [D, S]
    WT = Wqkv.T.astype(f16)                                          # [D, 8192]
    WoT = Wout.T.astype(f16)                                         # [D, D]

    # hiddenQ[qc, b, p, j, c] = hiddenT[128*(12b+j)+p, 512qc+c]
    hiddenQ = np.ascontiguousarray(
        hiddenT.reshape(4, 12, P, 4, QCW).transpose(3, 0, 2, 1, 4)
    )  # [qc, b, p, j, c]

    half = HD // 2
    inv = (1.0 / (500000.0 ** (np.arange(half, dtype=np.float32) * 2.0 / HD)))
    ang = pos.astype(np.float32)[:, None] * inv[None, :].astype(np.float32)
    cos = np.cos(ang).T.astype(np.float32)                           # [64, S]
    sin = np.sin(ang).T.astype(np.float32)
    cc = np.concatenate([cos, cos], axis=0)                          # [128, S]
    ss = np.concatenate([-sin, sin], axis=0)
    ccq = np.ascontiguousarray((cc * SCALE).astype(f16))
    ssq = np.ascontiguousarray((ss * SCALE).astype(f16))
    cck = np.ascontiguousarray(cc.astype(f16))
    ssk = np.ascontiguousarray(ss.astype(f16))
    idx = np.arange(P)
    identm = np.eye(P, dtype=np.float16)
    maskdm = np.where(idx[None, :] > idx[:, None], -60000.0, 0.0).astype(np.float16)

    in_maps = []
    for c in range(N_CORES):
        # per-core wqkv columns, pass-major: A = [k, v, q0, q1], B = [q2..q5]
        kcol = WT[:, D + P * c:D + P * (c + 1)]
        vcol = WT[:, D + 1024 + P * c:D + 1024 + P * (c + 1)]
        qcols = [WT[:, 768 * c + P * m:768 * c + P * (m + 1)]
                 for m in range(6)]
        wA = np.concatenate([kcol, vcol, qcols[0], qcols[1]], axis=1)
        wB = np.concatenate(qcols[2:6], axis=1)
        # [blk, p, j, c]: row 128*(4blk+j)+p
        wqkvA = np.ascontiguousarray(
            wA.reshape(12, 4, P, 512).transpose(0, 2, 1, 3))
        wqkvB = np.ascontiguousarray(
            wB.reshape(12, 4, P, 512).transpose(0, 2, 1, 3))
        # woG[g, p, h, c] = WoT[768c + 128h + p, 512g + c]
        wo = WoT[768 * c:768 * (c + 1), :]                           # [768, D]
        woGm = np.ascontiguousarray(
            wo.reshape(NQH, P, 12, 512).transpose(2, 1, 0, 3))
        in_maps.append(dict(hiddenQ=hiddenQ, wqkvA=wqkvA, wqkvB=wqkvB,
                            woG=woGm, ccq=ccq, ssq=ssq, cck=cck, ssk=ssk,
                            ident=identm, maskd=maskdm))

    nc = _get_nc()
    res = run_bass_kernel_spmd(nc, in_maps, core_ids=list(range(N_CORES)))
    kernel._last_results = res

    # unshard: each core returns its partial out-projection (contraction over
    # its 6 heads); the full output is the sum of the 8 partials.
    outT = np.zeros((D, S), np.float32)
    for qc in range(NQC):
        acc = np.zeros((D, QCW), np.float32)
        for c in range(N_CORES):
            acc += res.results[c][f"out{qc}"].astype(np.float32)
        outT[:, QCW * qc:QCW * (qc + 1)] = acc
    return np.ascontiguousarray(outT.T)[None]


# revision 17
# speedup vs baseline: 1.0709x; 1.0182x over previous
"""DbrxAttention (B=1, S=2048, D=6144, 48 q heads / 8 kv heads, rope, causal)
on 8 Trainium2 NeuronCores.

Sharding: tensor-parallel across heads. Core c owns q heads [6c, 6c+6) and kv
head c (GQA groups align). Each core computes its partial out-projection
(contraction over its 6 heads) for the full [D, S] output; the host sums the
8 f16 partials in fp32. No on-device collective: profiling showed each
ReduceScatter ran ~90us on the CC cores and its completion-wait at the head
of the sync/gpsimd queues starved stage-3 DMAs for 25-55us per chunk.

Structure (from iterative neuron-profile analysis; every >3.4us PE gap
re-throttles the PE clock to 1.2GHz, so the whole design is about keeping
the matmul stream dense):
- Stage 1 is two passes of 4 PSUM banks: pass A {k, v, q0, q1}, pass B
  {q2..q5}. Rope for k/q0/q1 and the V transpose run under pass B, so
  scores start immediately after pass B. Hidden activations are per-chunk
  resident (4 quarter tiles, 4-deep ring); weights stream from pass-major
  restaged DRAM in 512KB blocks. Pass-B triggers are split around the rope
  swaps (4 before, 8 after) so neither starves the other on the gpsimd
  queue; hq/tab triggers interleave into the pass-A block stream.
- All psum tiles come from one 8-buffer ring of [128,512] f32 tiles; ring
  order = emission order. Buffers whose readers would be emitted more than
  8 allocations after the slot recycles are NOT safe (the allocator
  registers slot-reuse dependencies at allocation time) - this constrains
  PV to lag scores by exactly one head with probsT bufs=2.
- Softmax flow: per-512 exp with accumulator sidecar (fp32, constant shift
  exp(s-12)), fp32 normalize, f16 cast, then xbar transposes alternating
  between the sync and scalar HWDGE queues (a single queue serializes on
  the ~2.5us completion latency). probs16/p32 rings 5/6 deep keep the
  mul->transpose chain from re-serializing.
- Stage 3 streams wout per chunk in 12x768KB blocks and writes 4-dm-tile
  batched stores straight to the per-chunk output tensor via a rearranged
  dest AP.

Numerics: f16 matmul operands, fp32 PSUM accumulation, fp32 softmax with a
constant shift (max score ~21.5; the spread between global max and smallest
row max is ~26 nats so the normalize must stay fp32), f16 rope
(rel err ~1.4e-3 overall). Measured: 999.9us vs 1285.3us baseline.
"""

import numpy as np

N_CORES = 8
S = 2048
D = 6144
HD = 128
NQH = 6                 # q heads per core
P = 128
NKT = S // P            # 16 key tiles
NQC = 4                 # q chunks
QCW = S // NQC          # 512
DT = D // P             # 48 d-model tiles
SCALE = HD ** -0.5
CAP = 12.0              # softmax constant shift
CLIP = 8.0

_cached_nc = None


def _build_nc():
    import concourse.mybir as mybir
    import concourse.tile as tile
    from concourse import bacc

    f16, f32 = mybir.dt.float16, mybir.dt.float32
    add_op = mybir.AluOpType.add
    mult_op = mybir.AluOpType.mult
    min_op = mybir.AluOpType.min
    max_op = mybir.AluOpType.max
    X = mybir.AxisListType.X
    Exp = mybir.ActivationFunctionType.Exp

    nc = bacc.Bacc("TRN2", target_bir_lowering=False, debug=False,
                   num_devices=N_CORES)

    # hiddenQ[qc, b, p, j, c] = hiddenT[128*(12b+j)+p, 512qc+c]
    hiddenQ = nc.dram_tensor("hiddenQ", [NQC, 4, P, 12, QCW], f16,
                             kind="ExternalInput").ap()
    # wqkvA/B[blk, p, j, c]: pass-major weight blocks, 4 kt per block,
    # c = 4 m-slices of 128 (A: k,v,q0,q1 / B: q2..q5)
    wqkvA = nc.dram_tensor("wqkvA", [12, P, 4, 512], f16,
                           kind="ExternalInput").ap()
    wqkvB = nc.dram_tensor("wqkvB", [12, P, 4, 512], f16,
                           kind="ExternalInput").ap()
    # woG[g, p, h, c] = woutT[128h+p, 512g+c]
    woG = nc.dram_tensor("woG", [12, P, NQH, 512], f16,
                         kind="ExternalInput").ap()
    ccq = nc.dram_tensor("ccq", [P, S], f16, kind="ExternalInput").ap()
    ssq = nc.dram_tensor("ssq", [P, S], f16, kind="ExternalInput").ap()
    cck = nc.dram_tensor("cck", [P, S], f16, kind="ExternalInput").ap()
    ssk = nc.dram_tensor("ssk", [P, S], f16, kind="ExternalInput").ap()
    ident = nc.dram_tensor("ident", [P, P], f16, kind="ExternalInput").ap()
    maskd = nc.dram_tensor("maskd", [P, P], f16, kind="ExternalInput").ap()
    outs = [nc.dram_tensor(f"out{g}", [D, QCW], f16,
                           kind="ExternalOutput").ap() for g in range(NQC)]

    with tile.TileContext(nc) as tc:
        with (
            tc.tile_pool(name="const", bufs=1) as const,
            tc.tile_pool(name="kv", bufs=1) as kvp,
            tc.tile_pool(name="stream", bufs=1) as stream,
            tc.tile_pool(name="work", bufs=1) as work,
            tc.tile_pool(name="stats", bufs=1) as stats,
            tc.tile_pool(name="ps", bufs=1, space="PSUM") as psp,
        ):
            ident_sb = const.tile([P, P], f16, tag="ident")
            nc.sync.dma_start(ident_sb[:], ident[:])
            maskd_sb = const.tile([P, P], f16, tag="maskd")
            nc.sync.dma_start(maskd_sb[:], maskd[:])
            negcap = const.tile([P, 1], f32, tag="negcap")
            nc.vector.memset(negcap[:], -CAP)
            # pre-warm the exp table so the 2.7us ACT_TABLE_LOAD is off the
            # chunk-0 softmax critical path
            warm = stats.tile([P, 1], f32, tag="warm", name="warm")
            nc.scalar.activation(warm[:], negcap[:], Exp, scale=1.0)

            k_sb = kvp.tile([P, S], f16, tag="k_sb")
            v_sb = kvp.tile([P, NKT, P], f16, tag="v_sb")

            def ring(nm):
                return psp.tile([P, QCW], f32, tag="pb", bufs=8, name=nm)

            for qc in range(NQC):
                cs = slice(QCW * qc, QCW * (qc + 1))

                # ---- chunk-resident hidden quarters + rope tables
                def hq_load(b):
                    hq = stream.tile([P, 12, QCW], f16, tag="hq", bufs=4,
                                     name=f"hq{b}")
                    nc.gpsimd.dma_start(hq[:], hiddenQ[qc, b])
                    return hq

                hqs = [hq_load(0)]

                def hrhs(kt):
                    return hqs[kt // 12][:, kt % 12, :]

                q_h = [work.tile([P, QCW], f16, tag=f"qh{m}", bufs=2,
                                 name=f"q{m}") for m in range(NQH)]

                # ---- rope helpers (clip runs ahead of its mult chain)
                def emit_clip_v(ps_v):
                    vT = work.tile([P, QCW], f16, tag="vT", bufs=1,
                                   name="vT")
                    nc.vector.tensor_scalar(
                        vT[:], ps_v[:], CLIP, -CLIP, min_op, max_op)
                    nc.sync.dma_start_transpose(
                        v_sb[:, 4 * qc:4 * (qc + 1), :], vT[:])

                def emit_clip(ps_m):
                    a_t = work.tile([P, QCW], f16, tag="ropeA", bufs=3,
                                    name="a_t")
                    nc.vector.tensor_scalar(
                        a_t[:], ps_m[:], CLIP, -CLIP, min_op, max_op)
                    b_t = work.tile([P, QCW], f16, tag="ropeB", bufs=3,
                                    name="b_t")
                    nc.gpsimd.dma_start(b_t[0:64, :], a_t[64:128, :])
                    nc.gpsimd.dma_start(b_t[64:128, :], a_t[0:64, :])
                    return (a_t, b_t)

                def emit_chain(m, ab):
                    # m == -1 means the k head
                    a_t, b_t = ab
                    cc_t = tabs["cck"] if m < 0 else tabs["ccq"]
                    ss_t = tabs["ssk"] if m < 0 else tabs["ssq"]
                    e_t = work.tile([P, QCW], f16, tag="ropeE", bufs=2,
                                    name="e_t")
                    nc.vector.tensor_tensor(e_t[:], a_t[:], cc_t[:], mult_op)
                    nc.vector.tensor_tensor(b_t[:], b_t[:], ss_t[:], mult_op)
                    dst = k_sb[:, cs] if m < 0 else q_h[m][:]
                    nc.vector.tensor_tensor(dst, e_t[:], b_t[:], add_op)

                # ---- stage 1 pass A: {k, v, q0, q1}
                psA = [ring(f"A{i}") for i in range(4)]
                tabs = {}
                for blk in range(12):
                    wa_t = stream.tile([P, 4, 512], f16, tag="wa", bufs=2,
                                       name="wa_t")
                    nc.gpsimd.dma_start(wa_t[:], wqkvA[blk])
                    if blk in (1, 4, 7):
                        hqs.append(hq_load(len(hqs)))
                    elif blk == 8:
                        for nm, tsrc in (("ccq", ccq), ("ssq", ssq),
                                         ("cck", cck), ("ssk", ssk)):
                            t = stream.tile([P, QCW], f16, tag=nm, bufs=2,
                                            name=nm)
                            nc.gpsimd.dma_start(t[:], tsrc[:, cs])
                            tabs[nm] = t
                    for j in range(4):
                        kt = 4 * blk + j
                        for mi in range(4):
                            nc.tensor.matmul(
                                psA[mi][:], wa_t[:, j, P * mi:P * (mi + 1)],
                                hrhs(kt),
                                start=(kt == 0), stop=(kt == DT - 1))
                # pass-B weight triggers go on the gpsimd queue BEFORE the
                # rope work so the first blocks prefetch during pass A;
                # later triggers self-throttle on the wb ring.
                wb_ts = []
                for blk in range(6):
                    wb_t = stream.tile([P, 4, 512], f16, tag="wb", bufs=4,
                                       name="wb_t")
                    nc.gpsimd.dma_start(wb_t[:], wqkvB[blk])
                    wb_ts.append(wb_t)
                # drain A: clips first, then chains (k, q0, q1); v transpose
                ab_k = emit_clip(psA[0])
                emit_clip_v(psA[1])
                ab_q0 = emit_clip(psA[2])
                ab_q1 = emit_clip(psA[3])
                emit_chain(-1, ab_k)
                emit_chain(0, ab_q0)
                emit_chain(1, ab_q1)
                for blk in range(6, 12):
                    wb_t = stream.tile([P, 4, 512], f16, tag="wb", bufs=4,
                                       name="wb_t")
                    nc.gpsimd.dma_start(wb_t[:], wqkvB[blk])
                    wb_ts.append(wb_t)

                # ---- stage 1 pass B: {q2..q5}  (rope A runs under this)
                psB = [ring(f"B{i}") for i in range(4)]
                for blk in range(12):
                    wb_t = wb_ts[blk]
                    for j in range(4):
                        kt = 4 * blk + j
                        for mi in range(4):
                            nc.tensor.matmul(
                                psB[mi][:], wb_t[:, j, P * mi:P * (mi + 1)],
                                hrhs(kt),
                                start=(kt == 0), stop=(kt == DT - 1))
                abs_b = [emit_clip(psB[i]) for i in range(4)]
                for i in range(4):
                    emit_chain(2 + i, abs_b[i])

                # ---- stage 2: scores + softmax, PV lagging one head so the
                # PE never waits more than one softmax tail (and the probsT
                # bufs=2 ring stays acyclic across engines)
                njt = 4 * (qc + 1)
                attnT = work.tile([P, NQH, QCW], f16, tag="attnT", bufs=1,
                                  name="attnT")

                def emit_pv(h, probsT):
                    ps_pv = ring("pv")
                    for j in range(njt):
                        nc.tensor.matmul(
                            ps_pv[:], v_sb[:, j, :], probsT[:, j, :],
                            start=(j == 0), stop=(j == njt - 1))
                    nc.vector.tensor_copy(attnT[:, h, :], ps_pv[:])

                probsTs = {}
                for h in range(NQH):
                    probsT = work.tile([P, NKT, QCW], f16, tag="probsT",
                                       bufs=2, name="probsT")
                    probsTs[h] = probsT
                    for jl in range(1, 4):
                        nc.vector.memset(
                            probsT[:, 4 * qc + jl, :P * jl], 0.0)
                    for il in range(4):
                        i = 4 * qc + il
                        L = P * (i + 1)
                        nkc = (L + 511) // 512
                        s_all = stats.tile([P, 4], f32, tag="s_all",
                                           bufs=3, name="s_all")
                        probs16 = work.tile([P, S], f16, tag="probs16",
                                            bufs=5, name="probs16")
                        pscs = []
                        for kc in range(nkc):
                            n = min(512, L - 512 * kc)
                            last = kc == nkc - 1
                            psc = ring("psc")
                            nc.tensor.matmul(
                                psc[:, :n],
                                q_h[h][:, P * il:P * (il + 1)],
                                k_sb[:, 512 * kc:512 * kc + n],
                                start=True, stop=not last)
                            if last:
                                nc.tensor.matmul(
                                    psc[:, n - P:n], ident_sb[:],
                                    maskd_sb[:], start=False, stop=True)
                            pscs.append((psc, n, kc))
                        p32s = []
                        for psc, n, kc in pscs:
                            p32 = work.tile([P, 512], f32, tag="p32",
                                            bufs=6, name="p32")
                            nc.scalar.activation(
                                p32[:, :n],
                                psc[:, :n], Exp, bias=negcap[:],
                                scale=1.0,
                                accum_out=s_all[:, kc:kc + 1])
                            p32s.append((p32, n, kc))
                        ssum = stats.tile([P, 1], f32, tag="ssum",
                                          bufs=3, name="ssum")
                        nc.vector.reduce_sum(ssum[:], s_all[:, :nkc],
                                             axis=X)
                        rcp = stats.tile([P, 1], f32, tag="rcp",
                                         bufs=3, name="rcp")
                        nc.vector.reciprocal(rcp[:], ssum[:])
                        for p32, n, kc in p32s:
                            nc.vector.tensor_scalar_mul(
                                probs16[:, 512 * kc:512 * kc + n],
                                p32[:, :n], rcp[:])
                        teng = nc.sync if il % 2 == 1 else nc.scalar
                        teng.dma_start_transpose(
                            probsT[:, :i + 1, P * il:P * (il + 1)],
                            probs16[:, :L])
                    if h >= 1:
                        emit_pv(h - 1, probsTs[h - 1])
                emit_pv(NQH - 1, probsTs[NQH - 1])

                # ---- stage 3: out-proj, batched stores direct to output
                outT_qc = outs[qc]
                for g in range(12):
                    wo_t = stream.tile([P, NQH, 512], f16, tag="wo", bufs=3,
                                       name="wo_t")
                    nc.gpsimd.dma_start(wo_t[:], woG[g])
                    ot = work.tile([P, 4, QCW], f16, tag="ot", bufs=2,
                                   name="ot")
                    for i in range(4):
                        pso = ring("pso")
                        for h6 in range(NQH):
                            nc.tensor.matmul(
                                pso[:],
                                wo_t[:, h6, P * i:P * (i + 1)],
                                attnT[:, h6, :],
                                start=(h6 == 0), stop=(h6 == NQH - 1))
                        nc.scalar.copy(ot[:, i, :], pso[:])
                    nc.gpsimd.dma_start(
                        outT_qc[512 * g:512 * (g + 1), :].rearrange(
                            "(i p) c -> p i c", p=P),
                        ot[:])

    nc.compile()
    return nc


def _get_nc():
    global _cached_nc
    if _cached_nc is None:
        _cached_nc = _build_nc()
    return _cached_nc


def kernel(**inputs):
    from concourse.bass_utils import run_bass_kernel_spmd

    hs = np.asarray(inputs["hidden_states"])[0].astype(np.float32)   # [S, D]
    Wqkv = np.asarray(inputs["Wqkv"]).astype(np.float32)             # [8192, D]
    Wout = np.asarray(inputs["Wout"]).astype(np.float32)             # [D, D]
    pos = np.asarray(inputs["position_ids"])[0]

    f16 = np.float16
    hiddenT = np.ascontiguousarray(hs.T).astype(f16)                 # [D, S]
    WT = Wqkv.T.astype(f16)                                          # [D, 8192]
    WoT = Wout.T.astype(f16)                                         # [D, D]

    # hiddenQ[qc, b, p, j, c] = hiddenT[128*(12b+j)+p, 512qc+c]
    hiddenQ = np.ascontiguousarray(
        hiddenT.reshape(4, 12, P, 4, QCW).transpose(3, 0, 2, 1, 4)
    )  # [qc, b, p, j, c]

    half = HD // 2
    inv = (1.0 / (500000.0 ** (np.arange(half, dtype=np.float32) * 2.0 / HD)))
    ang = pos.astype(np.float32)[:, None] * inv[None, :].astype(np.float32)
    cos = np.cos(ang).T.astype(np.float32)                           # [64, S]
    sin = np.sin(ang).T.astype(np.float32)
    cc = np.concatenate([cos, cos], axis=0)                          # [128, S]
    ss = np.concatenate([-sin, sin], axis=0)
    ccq = np.ascontiguousarray((cc * SCALE).astype(f16))
    ssq = np.ascontiguousarray((ss * SCALE).astype(f16))
    cck = np.ascontiguousarray(cc.astype(f16))
    ssk = np.ascontiguousarray(ss.astype(f16))
    idx = np.arange(P)
    identm = np.eye(P, dtype=np.float16)
    maskdm = np.where(idx[None, :] > idx[:, None], -60000.0, 0.0).astype(np.float16)

    in_maps = []
    for c in range(N_CORES):
        # per-core wqkv columns, pass-major: A = [k, v, q0, q1], B = [q2..q5]
        kcol = WT[:, D + P * c:D + P * (c + 1)]
        vcol = WT[:, D + 1024 + P * c:D + 1024 + P * (c + 1)]
        qcols = [WT[:, 768 * c + P * m:768 * c + P * (m + 1)]
                 for m in range(6)]
        wA = np.concatenate([kcol, vcol, qcols[0], qcols[1]], axis=1)
        wB = np.concatenate(qcols[2:6], axis=1)
        # [blk, p, j, c]: row 128*(4blk+j)+p
        wqkvA = np.ascontiguousarray(
            wA.reshape(12, 4, P, 512).transpose(0, 2, 1, 3))
        wqkvB = np.ascontiguousarray(
            wB.reshape(12, 4, P, 512).transpose(0, 2, 1, 3))
        # woG[g, p, h, c] = WoT[768c + 128h + p, 512g + c]
        wo = WoT[768 * c:768 * (c + 1), :]                           # [768, D]
        woGm = np.ascontiguousarray(
            wo.reshape(NQH, P, 12, 512).transpose(2, 1, 0, 3))
        in_maps.append(dict(hiddenQ=hiddenQ, wqkvA=wqkvA, wqkvB=wqkvB,
                            woG=woGm, ccq=ccq, ssq=ssq, cck=cck, ssk=ssk,
                            ident=identm, maskd=maskdm))

    nc = _get_nc()
    res = run_bass_kernel_spmd(nc, in_maps, core_ids=list(range(N_CORES)))
    kernel._last_results = res

    # unshard: each core returns its partial out-projection (contraction over
    # its 6 heads); the full output is the sum of the 8 partials.
    outT = np.zeros((D, S), np.float32)
    for qc in range(NQC):
        acc = np.zeros((D, QCW), np.float32)
        for c in range(N_CORES):
            acc += res.results[c][f"out{qc}"].astype(np.float32)
        outT[:, QCW * qc:QCW * (qc + 1)] = acc
    return np.ascontiguousarray(outT.T)[None]
